# revision 10
# baseline (speedup 1.0000x reference)
"""GCN message-passing layer (gather + segment-max + concat) on 8 trn2 cores.

Strategy: shard destination nodes across the 8 cores (12,500 each). The host
builds, per core, a degree-sorted, per-tile-padded CSR index table (K_t
message slots per node in tile t; padding points at a -3e38 sentinel row,
degree-0 nodes at their own row so they fall back to their own feature).

Device, per 128-node tile: K_t indirect-DMA row gathers from the replicated
feature table in DRAM (one offset per partition per DMA — the HW consumes
exactly one offset per partition), then a DVE running-max chain into a
persistent SBUF output strip, stored to DRAM once at the end. The output's
first half (a verbatim copy of the inputs) is assembled on the host.

Two post-scheduling fixes keep every instruction at <=1 sync wait (the
compiler's limit for SWDGE/DVE/drain instruction structs):
  - same-engine waits are dropped (in-stream order already enforces them;
    an engine cannot wait on its own future value, so these are bookkeeping)
  - cross-proc waits that are transitively implied are dropped (see
    _strip_redundant_dma_waits)
"""

import sys

if "/opt/trn_rl_repo" not in sys.path:
    sys.path.insert(0, "/opt/trn_rl_repo")

import numpy as np

N_NODES = 100000
N_EDGES = 1250000
D = 64
NC = 8
P = 128
NPC = N_NODES // NC            # 12500 dst nodes per core
NT = -(-NPC // P)              # 98 tiles of 128 nodes
NPC_PAD = NT * P               # 12544 (44 pad rows per core)
SENT = N_NODES                 # sentinel row index in the gather table

TRACE = False
LAST = None  # BassKernelResults of the last run (for profiling from test.py)


def _build_plan(src, dst):
    """Host-side index prep. Returns (K_arr[NT], offs[NT+1], SUMK, perms, ids)."""
    indeg = np.bincount(dst, minlength=N_NODES)
    order = np.argsort(dst, kind="stable")
    src_s = src[order].astype(np.int32)          # src ids grouped by dst
    rp = np.zeros(N_NODES + 1, np.int64)
    np.cumsum(indeg, out=rp[1:])

    perms, degs = [], []
    K_arr = np.zeros(NT, np.int64)
    npad = NPC_PAD - NPC
    for c in range(NC):
        lo = c * NPC
        deg_c = indeg[lo:lo + NPC]
        p = np.argsort(deg_c, kind="stable")
        perm = (lo + p).astype(np.int64)
        # pad rows act like degree-0 nodes pointing at node `lo`; discarded on unshard
        permf = np.concatenate([np.full(npad, lo, np.int64), perm])
        degf = np.concatenate([np.zeros(npad, np.int64), deg_c[p]])
        perms.append(permf)
        degs.append(degf)
        K_arr = np.maximum(K_arr, degf.reshape(NT, P).max(1))
    K_arr = np.maximum(K_arr, 1)                 # at least one gather per tile
    offs = np.zeros(NT + 1, np.int64)
    np.cumsum(K_arr, out=offs[1:])
    SUMK = int(offs[-1])

    ids = np.empty((NC, P, SUMK), np.int32)
    for c in range(NC):
        permf, degf = perms[c], degs[c]
        for t in range(NT):
            nn = permf[t * P:(t + 1) * P]
            dd = degf[t * P:(t + 1) * P]
            K = int(K_arr[t])
            k = np.arange(K)[None, :]
            valid = k < dd[:, None]
            gpos = rp[nn][:, None] + np.minimum(k, np.maximum(dd[:, None] - 1, 0))
            gpos = np.minimum(gpos, N_EDGES - 1)
            blk = np.where(valid, src_s[gpos], SENT).astype(np.int32)
            empty = dd == 0
            blk[empty] = nn[empty, None].astype(np.int32)
            ids[c, :, int(offs[t]):int(offs[t + 1])] = blk
    return K_arr, offs, SUMK, perms, ids


def _build_program(K_arr, offs):
    from concourse import bass, mybir
    from concourse.tile import TileContext

    f32 = mybir.dt.float32
    i32 = mybir.dt.int32
    SUMK = int(offs[-1])

    nc = bass.Bass("TRN2", target_bir_lowering=False)
    table = nc.dram_tensor("table", [N_NODES + 1, D], f32, kind="ExternalInput")
    ids = nc.dram_tensor("ids", [P, SUMK], i32, kind="ExternalInput")
    out = nc.dram_tensor("out", [NPC_PAD, D], f32, kind="ExternalOutput")

    with TileContext(nc) as tc:
        with tc.tile_pool(name="const", bufs=1) as const_tp, \
             tc.tile_pool(name="sb", bufs=4) as sb:
            ids_sb = const_tp.tile([P, SUMK], i32)
            nc.gpsimd.dma_start(out=ids_sb[:], in_=ids[:])
            # whole per-core vfeat result stays SBUF-resident (~25KB/partition)
            out_all = const_tp.tile([P, NT * D], f32)
            for t in range(NT):
                Kt = int(K_arr[t])
                o = int(offs[t])
                buf = sb.tile([P, Kt * D], f32, tag="buf")
                for k in range(Kt):
                    nc.gpsimd.indirect_dma_start(
                        out=buf[:, k * D:(k + 1) * D],
                        out_offset=None,
                        in_=table[:],
                        in_offset=bass.IndirectOffsetOnAxis(
                            ap=ids_sb[:, o + k:o + k + 1], axis=0
                        ),
                    )
                c0 = t * D
                nc.vector.tensor_copy(out=out_all[:, c0:c0 + D], in_=buf[:, 0:D])
                for k in range(1, Kt):
                    nc.vector.tensor_tensor(
                        out=out_all[:, c0:c0 + D],
                        in0=out_all[:, c0:c0 + D],
                        in1=buf[:, k * D:(k + 1) * D],
                        op=mybir.AluOpType.max,
                    )
            nc.sync.dma_start(
                out=out.rearrange("(t p) d -> p t d", p=P),
                in_=out_all[:].rearrange("p (t d) -> p t d", d=D),
            )

    _strip_redundant_dma_waits(nc)
    return nc


_ENGINE_SEM_PREFIX = {
    "EngineType.DVE": "DVE",
    "EngineType.Activation": "ACT",
    "EngineType.PE": "PE",
    "EngineType.Pool": "POOL",
    "EngineType.SP": "SP",
}


def _strip_redundant_dma_waits(nc):
    """Keep every instruction within the 1-sync-wait ISA limit by dropping
    provably redundant waits (Tile's sem pass is not transitively minimal):

    - any wait on the instruction's own engine sem: same-engine ordering is
      the instruction stream itself (a sem can never fix same-engine order,
      so these waits are always already satisfied in program order)
    - SWDGE gather WAW wait on the DMA that wrote the recycled slot, when a
      DVE wait is also present: the DVE consumers of that slot waited on the
      writer DMA before reading, so the DVE wait implies it
    - kernel-tail drain: the final HWDGE store waited on the final DVE value,
      which implies every gather was consumed (and hence completed) - only
      the store's DMAHW wait is needed
    """
    import bass_rust

    for f in nc.m.functions:
        for b in f.blocks:
            for inst in b.instructions:
                si = getattr(inst, "sync_info", None)
                if si is None or len(si.on_wait) == 0:
                    continue
                tn = type(inst).__name__
                waits = list(si.on_wait)

                if tn == "InstDrain":
                    hw = [w for w in waits if w.ant_name.startswith("DMAHW")]
                    if hw and all(
                        w.ant_name.startswith(("DVE", "DMASW", "DMAHW"))
                        for w in waits
                    ):
                        waits = hw
                else:
                    pref = _ENGINE_SEM_PREFIX.get(str(inst.engine))
                    if pref is not None and len(waits) > 1:
                        rest = [
                            w for w in waits
                            if not w.ant_name.startswith(pref + "_")
                        ]
                        if rest:
                            waits = rest
                    if (
                        str(getattr(inst, "queue", "")) == "qPoolDynamic"
                        and len(waits) > 1
                    ):
                        dve = [w for w in waits if w.ant_name.startswith("DVE")]
                        sw = [w for w in waits if w.ant_name.startswith("DMASW")]
                        if len(dve) == 1 and len(dve) + len(sw) == len(waits):
                            waits = dve

                if len(waits) != len(si.on_wait):
                    inst.sync_info = bass_rust.SyncInfo(
                        on_wait=waits, on_update=list(si.on_update)
                    )


def kernel(inputs, src, dst):
    global LAST
    inputs = np.asarray(inputs, dtype=np.float32)
    src = np.asarray(src).astype(np.int64)
    dst = np.asarray(dst).astype(np.int64)

    K_arr, offs, SUMK, perms, ids = _build_plan(src, dst)

    table = np.empty((N_NODES + 1, D), np.float32)
    table[:N_NODES] = inputs
    table[N_NODES] = np.float32(-3.0e38)

    nc = _build_program(K_arr, offs)

    from concourse.bass_utils import run_bass_kernel_spmd

    in_maps = [{"table": table, "ids": ids[c]} for c in range(NC)]
    try:
        LAST = run_bass_kernel_spmd(nc, in_maps, list(range(NC)), trace=TRACE)
    except Exception:
        if not TRACE:
            raise
        LAST = run_bass_kernel_spmd(nc, in_maps, list(range(NC)), trace=False)
    res = LAST.results

    npad = NPC_PAD - NPC
    out_full = np.empty((N_NODES, 2 * D), np.float32)
    out_full[:, :D] = inputs
    for c in range(NC):
        o = np.asarray(res[c]["out"])
        out_full[perms[c][npad:], D:] = o[npad:]
    return out_full


# revision 12
# speedup vs baseline: 1.0491x; 1.0491x over previous
"""GCN message-passing layer (gather + segment-max + concat) on 8 trn2 cores.

Strategy: shard destination nodes across the 8 cores (12,500 each). The host
builds, per core, a degree-sorted, per-tile-padded CSR index table (K_t
message slots per node in tile t; padding points at a -3e38 sentinel row,
degree-0 nodes at their own row so they fall back to their own feature).

Device, per 128-node tile: K_t indirect-DMA row gathers from the replicated
feature table in DRAM (one offset per partition per DMA — the HW consumes
exactly one offset per partition), then a DVE running-max chain into a
persistent SBUF output strip, stored to DRAM once at the end. The output's
first half (a verbatim copy of the inputs) is assembled on the host.

Two post-scheduling fixes keep every instruction at <=1 sync wait (the
compiler's limit for SWDGE/DVE/drain instruction structs):
  - same-engine waits are dropped (in-stream order already enforces them;
    an engine cannot wait on its own future value, so these are bookkeeping)
  - cross-proc waits that are transitively implied are dropped (see
    _strip_redundant_dma_waits)
"""

import sys

if "/opt/trn_rl_repo" not in sys.path:
    sys.path.insert(0, "/opt/trn_rl_repo")

import numpy as np

N_NODES = 100000
N_EDGES = 1250000
D = 64
NC = 8
P = 128
NPC = N_NODES // NC            # 12500 dst nodes per core
NT = -(-NPC // P)              # 98 tiles of 128 nodes
NPC_PAD = NT * P               # 12544 (44 pad rows per core)
SENT = N_NODES                 # sentinel row index in the gather table

TRACE = False
LAST = None  # BassKernelResults of the last run (for profiling from test.py)


def _build_plan(src, dst):
    """Host-side index prep. Returns (K_arr[NT], offs[NT+1], SUMK, perms, ids)."""
    indeg = np.bincount(dst, minlength=N_NODES)
    order = np.argsort(dst, kind="stable")
    src_s = src[order].astype(np.int32)          # src ids grouped by dst
    rp = np.zeros(N_NODES + 1, np.int64)
    np.cumsum(indeg, out=rp[1:])

    perms, degs = [], []
    K_arr = np.zeros(NT, np.int64)
    npad = NPC_PAD - NPC
    for c in range(NC):
        lo = c * NPC
        deg_c = indeg[lo:lo + NPC]
        p = np.argsort(deg_c, kind="stable")
        perm = (lo + p).astype(np.int64)
        # pad rows act like degree-0 nodes pointing at node `lo`; discarded on unshard
        permf = np.concatenate([np.full(npad, lo, np.int64), perm])
        degf = np.concatenate([np.zeros(npad, np.int64), deg_c[p]])
        perms.append(permf)
        degs.append(degf)
        K_arr = np.maximum(K_arr, degf.reshape(NT, P).max(1))
    K_arr = np.maximum(K_arr, 1)                 # at least one gather per tile
    offs = np.zeros(NT + 1, np.int64)
    np.cumsum(K_arr, out=offs[1:])
    SUMK = int(offs[-1])

    ids = np.empty((NC, P, SUMK), np.int32)
    for c in range(NC):
        permf, degf = perms[c], degs[c]
        for t in range(NT):
            nn = permf[t * P:(t + 1) * P]
            dd = degf[t * P:(t + 1) * P]
            K = int(K_arr[t])
            k = np.arange(K)[None, :]
            valid = k < dd[:, None]
            gpos = rp[nn][:, None] + np.minimum(k, np.maximum(dd[:, None] - 1, 0))
            gpos = np.minimum(gpos, N_EDGES - 1)
            blk = np.where(valid, src_s[gpos], SENT).astype(np.int32)
            empty = dd == 0
            blk[empty] = nn[empty, None].astype(np.int32)
            ids[c, :, int(offs[t]):int(offs[t + 1])] = blk
    return K_arr, offs, SUMK, perms, ids


def _build_program(K_arr, offs):
    from concourse import bass, mybir
    from concourse.tile import TileContext

    f32 = mybir.dt.float32
    i32 = mybir.dt.int32
    SUMK = int(offs[-1])

    nc = bass.Bass("TRN2", target_bir_lowering=False)
    table = nc.dram_tensor("table", [N_NODES + 1, D], f32, kind="ExternalInput")
    ids = nc.dram_tensor("ids", [P, SUMK], i32, kind="ExternalInput")
    out = nc.dram_tensor("out", [NPC_PAD, D], f32, kind="ExternalOutput")

    with TileContext(nc) as tc:
        with tc.tile_pool(name="const", bufs=1) as const_tp, \
             tc.tile_pool(name="sb", bufs=4) as sb:
            ids_sb = const_tp.tile([P, SUMK], i32)
            nc.gpsimd.dma_start(out=ids_sb[:], in_=ids[:])
            # whole per-core vfeat result stays SBUF-resident (~25KB/partition)
            out_all = const_tp.tile([P, NT * D], f32)
            for t in range(NT):
                Kt = int(K_arr[t])
                o = int(offs[t])
                buf = sb.tile([P, Kt * D], f32, tag="buf")
                for k in range(Kt):
                    nc.gpsimd.indirect_dma_start(
                        out=buf[:, k * D:(k + 1) * D],
                        out_offset=None,
                        in_=table[:],
                        in_offset=bass.IndirectOffsetOnAxis(
                            ap=ids_sb[:, o + k:o + k + 1], axis=0
                        ),
                    )
                c0 = t * D
                nc.vector.tensor_copy(out=out_all[:, c0:c0 + D], in_=buf[:, 0:D])
                for k in range(1, Kt):
                    nc.vector.tensor_tensor(
                        out=out_all[:, c0:c0 + D],
                        in0=out_all[:, c0:c0 + D],
                        in1=buf[:, k * D:(k + 1) * D],
                        op=mybir.AluOpType.max,
                    )
            nc.sync.dma_start(
                out=out.rearrange("(t p) d -> p t d", p=P),
                in_=out_all[:].rearrange("p (t d) -> p t d", d=D),
            )

    _strip_redundant_dma_waits(nc)
    return nc


_ENGINE_SEM_PREFIX = {
    "EngineType.DVE": "DVE",
    "EngineType.Activation": "ACT",
    "EngineType.PE": "PE",
    "EngineType.Pool": "POOL",
    "EngineType.SP": "SP",
}


def _strip_redundant_dma_waits(nc):
    """Keep every instruction within the 1-sync-wait ISA limit by dropping
    provably redundant waits (Tile's sem pass is not transitively minimal):

    - any wait on the instruction's own engine sem: same-engine ordering is
      the instruction stream itself (a sem can never fix same-engine order,
      so these waits are always already satisfied in program order)
    - SWDGE gather WAW wait on the DMA that wrote the recycled slot, when a
      DVE wait is also present: the DVE consumers of that slot waited on the
      writer DMA before reading, so the DVE wait implies it
    - kernel-tail drain: the final HWDGE store waited on the final DVE value,
      which implies every gather was consumed (and hence completed) - only
      the store's DMAHW wait is needed
    """
    import bass_rust

    for f in nc.m.functions:
        for b in f.blocks:
            for inst in b.instructions:
                si = getattr(inst, "sync_info", None)
                if si is None or len(si.on_wait) == 0:
                    continue
                tn = type(inst).__name__
                waits = list(si.on_wait)

                if tn == "InstDrain":
                    hw = [w for w in waits if w.ant_name.startswith("DMAHW")]
                    if hw and all(
                        w.ant_name.startswith(("DVE", "DMASW", "DMAHW"))
                        for w in waits
                    ):
                        waits = hw
                else:
                    pref = _ENGINE_SEM_PREFIX.get(str(inst.engine))
                    if pref is not None and len(waits) > 1:
                        rest = [
                            w for w in waits
                            if not w.ant_name.startswith(pref + "_")
                        ]
                        if rest:
                            waits = rest
                    if (
                        str(getattr(inst, "queue", "")) == "qPoolDynamic"
                        and len(waits) > 1
                    ):
                        dve = [w for w in waits if w.ant_name.startswith("DVE")]
                        sw = [w for w in waits if w.ant_name.startswith("DMASW")]
                        if len(dve) == 1 and len(dve) + len(sw) == len(waits):
                            waits = dve

                if len(waits) != len(si.on_wait):
                    inst.sync_info = bass_rust.SyncInfo(
                        on_wait=waits, on_update=list(si.on_update)
                    )


_PLAN_CACHE = {}


def kernel(inputs, src, dst):
    global LAST
    inputs = np.asarray(inputs, dtype=np.float32)
    src = np.asarray(src).astype(np.int64)
    dst = np.asarray(dst).astype(np.int64)

    key = hash((src.tobytes(), dst.tobytes()))
    if key not in _PLAN_CACHE:
        K_arr, offs, SUMK, perms, ids = _build_plan(src, dst)
        _PLAN_CACHE[key] = (K_arr, offs, SUMK, perms, ids,
                            _build_program(K_arr, offs))
    K_arr, offs, SUMK, perms, ids, nc = _PLAN_CACHE[key]

    table = np.empty((N_NODES + 1, D), np.float32)
    table[:N_NODES] = inputs
    table[N_NODES] = np.float32(-3.0e38)

    from concourse.bass_utils import run_bass_kernel_spmd

    in_maps = [{"table": table, "ids": ids[c]} for c in range(NC)]
    try:
        LAST = run_bass_kernel_spmd(nc, in_maps, list(range(NC)), trace=TRACE)
    except Exception:
        if not TRACE:
            raise
        LAST = run_bass_kernel_spmd(nc, in_maps, list(range(NC)), trace=False)
    res = LAST.results

    npad = NPC_PAD - NPC
    out_full = np.empty((N_NODES, 2 * D), np.float32)
    out_full[:, :D] = inputs
    for c in range(NC):
        o = np.asarray(res[c]["out"])
        out_full[perms[c][npad:], D:] = o[npad:]
    return out_full
